# revision 1
# baseline (speedup 1.0000x reference)
"""Transformer-XL multi-head self-attention on 8 Trainium2 NeuronCores.

Sharding: core c handles batch b = c//4 and heads {2*(c%4), 2*(c%4)+1}
(data-parallel over B x tensor-parallel over heads). Each core produces a
partial [N, E] output (its heads' w_o contributions); the host sums the 4
partials per batch element.

The XL relative-position term BD[i,j] = (q_i+v)·BDk[j-i+N-1] is computed
without the rel_shift gather: since rel_embed rows are sin/cos of
f_e*(j-i-H), the angle-difference identities turn BD into a plain matmul
    BD^T = Psi @ UW
with Psi[c,j] = [sin f_e(j-H); cos f_e(j-H)] (a shape-derived constant) and
UW[c,i] a per-query rotation of (q_i+v)@w_kr — so the whole score matrix
S^T = K q̃^T + Psi UW accumulates in PSUM with contraction 64+512.

Everything runs in the transposed orientation (keys on partitions, queries
on the free dim): softmax needs no max-subtraction (scores are O(5)), and
the denominator comes for free from a ones-column appended to V in the
attn@V matmul.
"""

import sys

sys.path.insert(0, "/opt/trn_rl_repo")

import ml_dtypes
import numpy as np

import concourse.bass as bass
import concourse.mybir as mybir
from concourse import bacc
from concourse.masks import make_identity
from concourse.tile import TileContext

F32 = mybir.dt.float32
BF16 = mybir.dt.bfloat16
AF = mybir.ActivationFunctionType
ALU = mybir.AluOpType

B, N, H, E, NH, D = 2, 2048, 2048, 512, 8, 64
HpN = H + N  # 4096
P = 128
NKT = HpN // P  # 32 key tiles
NQC = N // 512  # 4 query chunks of 512
NEC = E // P  # 4 contraction chunks over E
HEADS_PER_CORE = 2
N_CORES = 8


def build_program():
    nc = bacc.Bacc("TRN2", target_bir_lowering=False, debug=False)

    axT_d = nc.declare_dram_parameter("axT", [E, HpN], BF16, isOutput=False)
    rot_d = nc.declare_dram_parameter("rot", [2 * E // 2, N], BF16, isOutput=False)
    psi_d = nc.declare_dram_parameter("psi", [NKT * 384, P], BF16, isOutput=False)
    sc_d = nc.declare_dram_parameter("sc", [2 * P, 96], BF16, isOutput=False)
    wq_d = nc.declare_dram_parameter("wq", [2 * E, D], BF16, isOutput=False)
    wkv_d = nc.declare_dram_parameter("wkv", [2 * E, 2 * D], BF16, isOutput=False)
    wkrT_d = nc.declare_dram_parameter("wkrT", [2 * D, E], BF16, isOutput=False)
    wo_d = nc.declare_dram_parameter("wo", [2 * D, E], BF16, isOutput=False)
    ub_d = nc.declare_dram_parameter("ub", [2 * D, 1], F32, isOutput=False)
    vb_d = nc.declare_dram_parameter("vb", [2 * D, 1], F32, isOutput=False)
    out_d = nc.declare_dram_parameter("out", [N, E], F32, isOutput=True)

    with TileContext(nc) as tc:
        with (
            tc.tile_pool(name="persist", bufs=1) as persist,
            tc.tile_pool(name="head", bufs=1) as head_pool,
            tc.tile_pool(name="stream", bufs=2) as stream,
            tc.tile_pool(name="exps", bufs=6) as exps,
            tc.tile_pool(name="psis", bufs=2) as psis,
            tc.tile_pool(name="scratch", bufs=1) as scratch,
            tc.tile_pool(name="dram", bufs=1, space="DRAM") as dram_pool,
            tc.tile_pool(name="ph", bufs=1, space="PSUM") as ph,
            tc.tile_pool(name="pr", bufs=4, space="PSUM") as pr,
        ):
            _pa_ctr = [0]
            _pa_opts = None

            def pa_psum(shape, name, dtype=F32):
                # phase-A psum slots: cycle prot(4) + bank0-3 (idle until
                # attention) for an effectively 8-deep rotation
                i = _pa_ctr[0] % 8
                _pa_ctr[0] += 1
                if i < 4:
                    return pr.tile(shape, dtype, tag="prot", name=name)
                return ph.tile(
                    [P, 1024 if dtype is BF16 else 512], dtype, tag=f"bank{i - 4}", name=name
                )[: shape[0], : shape[1]]

            # ---- per-head weights first (small DMAs ahead of the big axT
            # load so the first projection matmuls are not queue-blocked)
            W = {}
            for h in range(HEADS_PER_CORE):
                for nm, dd in (("wq", wq_d), ("wkv", wkv_d)):
                    wd = D if nm == "wq" else 2 * D
                    wt = head_pool.tile(
                        [P, NEC, wd], BF16, tag=f"{nm}{h}", name=f"{nm}{h}"
                    )
                    nc.scalar.dma_start(
                        wt[:],
                        dd[h * E : (h + 1) * E].rearrange("(c p) d -> p c d", p=P),
                    )
                    W[nm, h] = wt
                for nm, dd, dt_ in (
                    ("wkrT", wkrT_d, BF16),
                    ("wo", wo_d, BF16),
                    ("ub", ub_d, F32),
                    ("vb", vb_d, F32),
                ):
                    shp = [D, E] if dt_ is BF16 else [D, 1]
                    wt = head_pool.tile(shp, dt_, tag=f"{nm}{h}", name=f"{nm}{h}")
                    nc.scalar.dma_start(wt[:], dd[h * D : (h + 1) * D])
                    W[nm, h] = wt

            # ---- resident tensors (x^T loaded in 4 E-chunks so the first
            # projection matmuls start before the whole 4MB lands)
            axT = []
            for c in range(NEC):
                axc = persist.tile([P, HpN], BF16, tag=f"axT{c}", name=f"axT{c}")
                # x-half first: the q projection only reads columns H:
                nc.sync.dma_start(axc[:, H:], axT_d[c * P : (c + 1) * P, H:])
                axT.append(axc)
            for c in range(NEC):
                nc.sync.dma_start(axT[c][:, 0:H], axT_d[c * P : (c + 1) * P, 0:H])
            out_acc = persist.tile([P, N // P, E], F32, tag="out_acc")
            sc_s = persist.tile([P, 2, 96], BF16, tag="sc")
            nc.scalar.dma_start(sc_s[:], sc_d[:].rearrange("(t p) k -> p t k", p=P))
            identb = persist.tile([P, P], BF16, tag="identb")
            make_identity(nc, identb[:])

            # =================== phase A: both heads' projections ============
            qtT, qvT, UW, kT, vo, wo_all = [], [], [], [], [], []
            for h in range(HEADS_PER_CORE):
                wq_s = W["wq", h]
                wkv_s = W["wkv", h]
                wkrT_s = W["wkrT", h]
                wo_s = W["wo", h]
                wo_all.append(wo_s)
                ub_s = W["ub", h]
                vb_s = W["vb", h]

                # q projection: qT = (x @ wq)^T, then +u / +v biases
                qt = head_pool.tile([P, N], BF16, tag=f"qtT{h}", name=f"qtT{h}")
                qv = head_pool.tile([D, N], BF16, tag=f"qvT{h}", name=f"qvT{h}")
                for qc in range(NQC):
                    pq = pa_psum([D, 512], "pq")
                    for c in range(NEC):
                        nc.tensor.matmul(
                            pq[:],
                            wq_s[:, c, :],
                            axT[c][:, H + qc * 512 : H + (qc + 1) * 512],
                            start=(c == 0),
                            stop=(c == NEC - 1),
                        )
                    qs = slice(qc * 512, (qc + 1) * 512)
                    nc.vector.tensor_scalar_add(qt[0:D, qs], pq[:], ub_s[:])
                    nc.vector.tensor_scalar_add(qv[:, qs], pq[:], vb_s[:])
                nc.sync.dma_start(qt[D : 2 * D, :], qt[0:D, :])
                qtT.append(qt)
                qvT.append(qv)

                # UW: per-query rotation of qv @ w_kr (positional contraction rows)
                uw = head_pool.tile([P, 3, N], BF16, tag=f"UW{h}", name=f"UW{h}")
                nc.gpsimd.memset(uw[96:128, 2, :], 0.0)
                pend_pc = []

                def flush_pc():
                    for puS, puW, pqs in pend_pc:
                        pc = pa_psum([96, 512], "pc")
                        nc.tensor.matmul(
                            pc[:], sc_s[:, 0, :], puS[:], start=True, stop=False
                        )
                        nc.tensor.matmul(
                            pc[:], sc_s[:, 1, :], puW[:], start=False, stop=True
                        )
                        nc.scalar.copy(uw[0:96, 2, pqs], pc[:])
                    del pend_pc[:]

                for qc in range(NQC):
                    qs = slice(qc * 512, (qc + 1) * 512)
                    cosb = stream.tile([P, 2, 512], BF16, tag="cosb")
                    nc.scalar.dma_start(
                        cosb[:], rot_d[0:256, qs].rearrange("(e p) w -> p e w", p=P)
                    )
                    sinb = stream.tile([P, 2, 512], BF16, tag="sinb")
                    nc.scalar.dma_start(
                        sinb[:], rot_d[256:512, qs].rearrange("(e p) w -> p e w", p=P)
                    )
                    for half in range(2):
                        gA = pa_psum([P, 512], "gA")
                        nc.tensor.matmul(
                            gA[:],
                            wkrT_s[:, half * P : (half + 1) * P],
                            qv[:, qs],
                            start=True,
                            stop=True,
                        )
                        gB = pa_psum([P, 512], "gB")
                        nc.tensor.matmul(
                            gB[:],
                            wkrT_s[:, (2 + half) * P : (3 + half) * P],
                            qv[:, qs],
                            start=True,
                            stop=True,
                        )
                        # U chunk = G*cosb + Gc*sinb ; W chunk = Gc*cosb - G*sinb
                        # ACT drains PSUM to bf16; DVE multiplies at the bf16
                        # 2x rate; gpsimd (SBUF-only) does the add/sub
                        sA = stream.tile([P, 512], BF16, tag="sA")
                        sB = stream.tile([P, 512], BF16, tag="sB")
                        nc.scalar.copy(sA[:], gA[:])
                        nc.scalar.copy(sB[:], gB[:])
                        m1 = stream.tile([P, 512], BF16, tag="uwtmp")
                        m2 = stream.tile([P, 512], BF16, tag="uwtmp2")
                        m3 = stream.tile([P, 512], BF16, tag="uwtmp3")
                        m2b = stream.tile([P, 512], BF16, tag="uwtmp4")
                        nc.vector.tensor_mul(m1[:], sA[:], cosb[:, half])
                        nc.vector.tensor_mul(m2[:], sB[:], sinb[:, half])
                        nc.vector.tensor_mul(m3[:], sB[:], cosb[:, half])
                        nc.vector.tensor_mul(m2b[:], sA[:], sinb[:, half])
                        if half == 0:
                            flush_pc()
                            # fast freqs: straight into UW chunks 0/1
                            nc.gpsimd.tensor_add(uw[:, 0, qs], m1[:], m2[:])
                            nc.gpsimd.tensor_sub(uw[:, 1, qs], m3[:], m2b[:])
                        else:
                            # slow freqs: compress onto the Chebyshev basis
                            uS = stream.tile([P, 512], BF16, tag="uS")
                            uWt = stream.tile([P, 512], BF16, tag="uWt")
                            nc.gpsimd.tensor_add(uS[:], m1[:], m2[:])
                            nc.gpsimd.tensor_sub(uWt[:], m3[:], m2b[:])
                            pend_pc.append((uS, uWt, qs))
                UW.append(uw)

                flush_pc()

                # [k|v]^T = (all_x @ [wk|wv])^T in one pass: psum rows
                # 0-63 = k^T, rows 64-127 = v^T
                kt_t = head_pool.tile([P, HpN], BF16, tag=f"kT{h}", name=f"kT{h}")
                vT = head_pool.tile([P, HpN], BF16, tag=f"vT{h}", name=f"vT{h}")
                for kc in range(HpN // 512):
                    pk = pa_psum([P, 512], "pk")
                    for c in range(NEC):
                        nc.tensor.matmul(
                            pk[:],
                            wkv_s[:, c, :],
                            axT[c][:, kc * 512 : (kc + 1) * 512],
                            start=(c == 0),
                            stop=(c == NEC - 1),
                        )
                    nc.scalar.copy(kt_t[0:D, kc * 512 : (kc + 1) * 512], pk[0:D, :])
                    nc.vector.tensor_copy(
                        vT[D : 2 * D, kc * 512 : (kc + 1) * 512], pk[D : 2 * D, :]
                    )
                nc.sync.dma_start(kt_t[D : 2 * D, :], kt_t[0:D, :])
                kT.append(kt_t)

                # v with an appended ones column [128, 32, 65]: PE-transpose
                # each [64, 128] block of v^T (vT rows 64-127) to key-major
                vo_t = head_pool.tile([P, NKT, D + 1], BF16, tag=f"vo{h}", name=f"vo{h}")
                for kt in range(NKT):
                    pv = pa_psum([P, D], "pv", BF16)
                    nc.tensor.transpose(
                        pv[:],
                        vT[D : 2 * D, kt * P : (kt + 1) * P],
                        identb[D : 2 * D, D : 2 * D],
                    )
                    nc.scalar.copy(vo_t[:, kt, 0:D], pv[:])
                nc.vector.memset(vo_t[:, :, D : D + 1], 1.0)
                vo.append(vo_t)

            # =================== phase B: attention + output ================
            def emit_attn_pair(h, av, kt, pend):
                psi_s = psis.tile([P, 2, 3, P], BF16, tag="psi", name="psi")
                nc.sync.dma_start(
                    psi_s[:],
                    psi_d[kt * 384 : (kt + 2) * 384].rearrange(
                        "(k c p) j -> p k c j", p=P, k=2
                    ),
                )
                for qc in range(NQC):
                    qs = slice(qc * 512, (qc + 1) * 512)
                    psA = pr.tile([P, 512], F32, tag="prot", name="psA")
                    nc.tensor.matmul(
                        psA[:],
                        kT[h][0:D, kt * P : (kt + 1) * P],
                        qtT[h][0:D, qs],
                        start=True,
                        stop=False,
                        tile_position=(0, 0),
                    )
                    psB = pr.tile([P, 512], F32, tag="prot", name="psB")
                    nc.tensor.matmul(
                        psB[:],
                        kT[h][D : 2 * D, (kt + 1) * P : (kt + 2) * P],
                        qtT[h][D : 2 * D, qs],
                        start=True,
                        stop=False,
                        tile_position=(64, 0),
                    )
                    for c in range(3):
                        nc.tensor.matmul(
                            psA[:],
                            psi_s[:, 0, c, :],
                            UW[h][:, c, qs],
                            start=False,
                            stop=(c == 2),
                        )
                    for c in range(3):
                        nc.tensor.matmul(
                            psB[:],
                            psi_s[:, 1, c, :],
                            UW[h][:, c, qs],
                            start=False,
                            stop=(c == 2),
                        )
                    for pkt, pqc, pet in pend:
                        nc.tensor.matmul(
                            av[pqc][:],
                            vo[h][:, pkt, :],
                            pet[:],
                            start=(pkt == 0),
                            stop=(pkt == NKT - 1),
                        )
                    del pend[:]
                    etA = exps.tile([P, 512], BF16, tag="exp", name="etA")
                    nc.scalar.activation(etA[:], psA[:], AF.Exp, scale=0.125)
                    etB = exps.tile([P, 512], BF16, tag="exp", name="etB")
                    nc.scalar.activation(etB[:], psB[:], AF.Exp, scale=0.125)
                    pend.extend([(kt, qc, etA), (kt + 1, qc, etB)])

            def emit_av_flush(h, av, pend):
                for pkt, pqc, pet in pend:
                    nc.tensor.matmul(
                        av[pqc][:],
                        vo[h][:, pkt, :],
                        pet[:],
                        start=(pkt == 0),
                        stop=(pkt == NKT - 1),
                    )
                del pend[:]

            def emit_num_z(h, av):
                # copy numerators + denominator row to SBUF; launch the Z
                # transpose round-trip DMAs
                numT = head_pool.tile(
                    [D + 1, N], BF16, tag=f"numT{h}", name=f"numT{h}"
                )
                for qc in range(NQC):
                    qs = slice(qc * 512, (qc + 1) * 512)
                    nc.vector.tensor_copy(numT[:, qs], av[qc][:])
                zdram = dram_pool.tile(
                    [1, N], BF16, tag=f"zdram{h}", name=f"zdram{h}"
                )
                nc.sync.dma_start(zdram[:], numT[D : D + 1, :])
                zT = scratch.tile([N // P, P], BF16, tag=f"zT{h}", name=f"zT{h}")
                nc.sync.dma_start(
                    zT[:], zdram[:].rearrange("a (s p) -> (a s) p", p=P)
                )
                return numT, zT

            def emit_out(h, numT, zT):
                # first 3 projection matmuls go ahead of the Z transpose so
                # the round-trip latency is covered without exhausting the 4
                # rotating PSUM slots (po0-2 + pz)
                pos = {}
                for s in range(3):
                    po = pr.tile([P, E], F32, tag="prot", name="po")
                    nc.tensor.matmul(
                        po[:],
                        numT[0:D, s * P : (s + 1) * P],
                        wo_all[h][:],
                        start=True,
                        stop=True,
                    )
                    pos[s] = po
                pz = pr.tile([P, N // P], BF16, tag="prot", name="pz")
                nc.tensor.transpose(pz[:], zT[:], identb[: N // P, : N // P])
                zrec = scratch.tile([P, N // P], F32, tag=f"zrec{h}", name=f"zrec{h}")
                nc.vector.reciprocal(zrec[:], pz[:])

                def scale_store(s, po):
                    if h == 0:
                        nc.vector.tensor_scalar_mul(
                            out_acc[:, s, :], po[:], zrec[:, s : s + 1]
                        )
                    else:
                        nc.vector.scalar_tensor_tensor(
                            out_acc[:, s, :],
                            po[:],
                            zrec[:, s : s + 1],
                            out_acc[:, s, :],
                            ALU.mult,
                            ALU.add,
                        )
                        nc.sync.dma_start(
                            out_d[:].rearrange("(s p) e -> p s e", p=P)[:, s, :],
                            out_acc[:, s, :],
                        )

                for s in range(3):
                    scale_store(s, pos[s])
                for s in range(3, N // P):
                    po = pr.tile([P, E], F32, tag="prot", name="po")
                    nc.tensor.matmul(
                        po[:],
                        numT[0:D, s * P : (s + 1) * P],
                        wo_all[h][:],
                        start=True,
                        stop=True,
                    )
                    scale_store(s, po)

            # h0 attention
            av0 = [
                ph.tile([D + 1, 512], F32, tag=f"bank{qc}", name=f"av0{qc}")
                for qc in range(NQC)
            ]
            pend0 = []
            for kt in range(0, NKT, 2):
                emit_attn_pair(0, av0, kt, pend0)
            emit_av_flush(0, av0, pend0)
            numT0, zT0 = emit_num_z(0, av0)
            # h1's first pair is emitted before h0's output projection so its
            # matmuls hide h0's Z round-trip latency on the PE
            av1 = [
                ph.tile([D + 1, 512], F32, tag=f"bank{qc}", name=f"av1{qc}")
                for qc in range(NQC)
            ]
            pend1 = []
            emit_attn_pair(1, av1, 0, pend1)
            emit_out(0, numT0, zT0)
            for kt in range(2, NKT, 2):
                emit_attn_pair(1, av1, kt, pend1)
            emit_av_flush(1, av1, pend1)
            numT1, zT1 = emit_num_z(1, av1)
            emit_out(1, numT1, zT1)

    nc.compile()
    return nc


_NC_CACHE = None


def _get_program():
    global _NC_CACHE
    if _NC_CACHE is None:
        _NC_CACHE = build_program()
    return _NC_CACHE


def make_in_maps(x, history, w_q, w_k, w_v, w_kr, w_o, u_bias, v_bias):
    all_x = np.concatenate([history, x], axis=1)  # [B, HpN, E]

    inv_freq = 1.0 / (10000.0 ** (np.arange(0, E, 2, dtype=np.float64) / E))  # [256]
    # fast half (e<128): exact sin/cos psi rows. slow half (e>=128, |angle|
    # <= 20.5 rad): compressed onto a shared 96-term Chebyshev basis in j
    # (lstsq fit, residual ~4e-14); the per-query coefficients are produced
    # on-device by two matmuls against `sc`.
    ang_f = np.outer(inv_freq[:128], np.arange(HpN, dtype=np.float64) - H)
    xn = (np.arange(HpN, dtype=np.float64) - H) / 2048.0
    T = np.polynomial.chebyshev.chebvander(xn, 95)  # [HpN, 96]
    ang_s = np.outer(xn * 2048.0, inv_freq[128:256])  # [HpN, 128]
    tgt = np.concatenate([np.sin(ang_s), np.cos(ang_s)], axis=1)  # [HpN, 256]
    coef, *_ = np.linalg.lstsq(T, tgt, rcond=None)  # [96, 256]
    sc = np.ascontiguousarray(coef.T)  # [256, 96]: rows 0-127 sin, 128-255 cos
    psi = np.concatenate(
        [np.sin(ang_f), np.cos(ang_f), T.T, np.zeros((32, HpN))], axis=0
    ).astype(np.float32)  # [384, HpN]
    psi = np.ascontiguousarray(
        psi.reshape(3, P, NKT, P).transpose(2, 0, 1, 3)
    ).reshape(NKT * 384, P)  # rows: kt*384 + c*128 + p
    ang_b = np.outer(inv_freq, np.arange(N, dtype=np.float64))  # [256, N]
    rot = np.ascontiguousarray(
        np.stack([np.cos(ang_b), np.sin(ang_b)]).astype(ml_dtypes.bfloat16).reshape(2 * E // 2, N)
    )

    in_maps = []
    for c in range(N_CORES):
        b = c // 4
        h0 = HEADS_PER_CORE * (c % 4)
        hs = slice(h0, h0 + HEADS_PER_CORE)
        bf = ml_dtypes.bfloat16
        axT = np.ascontiguousarray(all_x[b].T).astype(bf)
        in_maps.append(
            {
                "axT": axT,
                "rot": rot,
                "psi": psi.astype(bf),
                "sc": sc.astype(bf),
                "wq": np.ascontiguousarray(w_q[hs].reshape(2 * E, D)).astype(bf),
                "wkv": np.ascontiguousarray(
                    np.concatenate([w_k[hs], w_v[hs]], axis=-1).reshape(2 * E, 2 * D)
                ).astype(bf),
                "wkrT": np.ascontiguousarray(w_kr[hs].transpose(0, 2, 1))
                .reshape(2 * D, E)
                .astype(bf),
                "wo": np.ascontiguousarray(w_o[hs]).reshape(2 * D, E).astype(bf),
                "ub": np.ascontiguousarray(u_bias[hs].reshape(2 * D, 1)),
                "vb": np.ascontiguousarray(v_bias[hs].reshape(2 * D, 1)),
            }
        )
    return in_maps


def run(inputs, trace=False, **kw):
    from concourse.bass_utils import run_bass_kernel_spmd

    nc = _get_program()
    in_maps = make_in_maps(
        np.asarray(inputs["x"], np.float32),
        np.asarray(inputs["history"], np.float32),
        np.asarray(inputs["w_q"], np.float32),
        np.asarray(inputs["w_k"], np.float32),
        np.asarray(inputs["w_v"], np.float32),
        np.asarray(inputs["w_kr"], np.float32),
        np.asarray(inputs["w_o"], np.float32),
        np.asarray(inputs["u_bias"], np.float32),
        np.asarray(inputs["v_bias"], np.float32),
    )
    res = run_bass_kernel_spmd(nc, in_maps, list(range(N_CORES)), trace=trace, **kw)
    out = np.zeros((B, N, E), np.float32)
    for c in range(N_CORES):
        out[c // 4] += res.results[c]["out"].reshape(N, E)
    return out, res


def kernel(**inputs):
    # mask is all ones (per the problem spec), so score masking is a no-op
    # and the tensor is ignored.
    out, _ = run(inputs, trace=False)
    return out



# revision 36
# speedup vs baseline: 1.4003x; 1.4003x over previous
"""Transformer-XL multi-head self-attention on 8 Trainium2 NeuronCores.

Sharding: core c handles batch b = c//4 and heads {2*(c%4), 2*(c%4)+1}
(data-parallel over B x tensor-parallel over heads). Each core produces a
partial [N, E] output (its heads' w_o contributions); the host sums the 4
partials per batch element.

The XL relative-position term BD[i,j] = (q_i+v)·BDk[j-i+N-1] is computed
without the rel_shift gather via per-query rotation (angle-difference
identities): BD^T = Psi @ UW with Psi a shape-derived constant basis
(128 exact sin rows + 128 exact cos rows + 64 Chebyshev rows for the slow
frequencies) and UW per-query rotated coefficients.

Scores run on the PE in fp8e4 DoubleRow mode (0.5 cycles/row in the cost
model) with hi/lo error compensation: a bf16-accurate operand x is split
as x = hi + lo with hi = fp8(x), lo = fp8(x - hi), keeping the hi*hi +
hi*lo + lo*hi cross terms. Per 128-key tile the contraction is 7 chunks of
128 rows consumed by 4 DoubleRow calls:
  [sin|cos]x[Uhi|Whi], [sin|cos]x[Ulo|Wlo],
  [khi|Thi]x[qhi|chi], [klo|Thi]x[qhi|clo], [khi|Tlo]x[qlo|chi], pad
where T/c are the Chebyshev basis/coefficients and k/q carry the content
term (q+u)·k. Measured quantization noise: ~0.1% (AC, cheb) / ~0.9%
(fast psi, one-sided) versus 2.4-3% for direct fp8. The value path
(exp, V, attn@V, output projection) stays in bf16: fp8 noise there does
not average out (the attention output is a random-walk sum, so per-stage
relative error passes through at full strength).
"""

import sys

sys.path.insert(0, "/opt/trn_rl_repo")

import ml_dtypes
import numpy as np

import concourse.bass as bass
import concourse.mybir as mybir
from concourse import bacc
from concourse.masks import make_identity
from concourse.tile import TileContext

F32 = mybir.dt.float32
BF16 = mybir.dt.bfloat16
FP8 = mybir.dt.float8e4
I16 = mybir.dt.int16
AF = mybir.ActivationFunctionType
ALU = mybir.AluOpType
DR = mybir.MatmulPerfMode.DoubleRow

B, N, H, E, NH, D = 2, 2048, 2048, 512, 8, 64
HpN = H + N  # 4096
P = 128
NKT = HpN // P  # 32 key tiles
NPAIR = NKT // 2  # 16 key-tile pairs
NQC = N // 512  # 4 query chunks of 512
NEC = E // P  # 4 contraction chunks over E
NS = N // P  # 16 output row tiles
NT = 64  # chebyshev terms
HEADS_PER_CORE = 2
N_CORES = 8

LOG2E = 1.4426950408889634
SCORE_SHIFT = 1.5  # exp(s - c): cancels in softmax, bounds exp values
# fraction of exp tiles on DVE (Schraudolph, lossier): (ctr % MOD) < DVE
EXP_MOD, EXP_DVE = 3, 1


def build_program():
    nc = bacc.Bacc("TRN2", target_bir_lowering=False, debug=False)

    axT_d = nc.declare_dram_parameter("axT", [E, HpN], BF16, isOutput=False)
    rot_d = nc.declare_dram_parameter("rot", [E, N], BF16, isOutput=False)
    # SgF: shared fast-psi chunks [kt, 2, p, j] = [sin_hi, cos_hi]
    psiF_d = nc.declare_dram_parameter("psiF", [NKT * 2 * P, P], FP8, isOutput=False)
    # SgA per head: [kt, 4, p, j]; k-half zeros (filled on device), T-half host
    psiA_d = nc.declare_dram_parameter(
        "psiA", [HEADS_PER_CORE * NKT * 4 * P, P], FP8, isOutput=False
    )
    sc_d = nc.declare_dram_parameter("sc", [2 * P, NT], BF16, isOutput=False)
    wq2_d = nc.declare_dram_parameter("wq2", [E, P], BF16, isOutput=False)
    wk2_d = nc.declare_dram_parameter("wk2", [E, P], BF16, isOutput=False)
    wv2_d = nc.declare_dram_parameter("wv2", [E, P], BF16, isOutput=False)
    wkrT_d = nc.declare_dram_parameter("wkrT", [P, E], BF16, isOutput=False)
    wo2_d = nc.declare_dram_parameter("wo2", [D, 2 * E], BF16, isOutput=False)
    ub2_d = nc.declare_dram_parameter("ub2", [P, 1], F32, isOutput=False)
    vb2_d = nc.declare_dram_parameter("vb2", [P, 1], F32, isOutput=False)
    out_d = nc.declare_dram_parameter("out", [N, E], BF16, isOutput=True)

    with TileContext(nc) as tc:
        with (
            tc.tile_pool(name="persist", bufs=1) as persist,
            tc.tile_pool(name="gst", bufs=2) as gst,       # G copies stream
            tc.tile_pool(name="mst", bufs=2) as mst,       # rotation temps
            tc.tile_pool(name="est", bufs=6) as est,       # exp tiles
            tc.tile_pool(name="dram", bufs=1, space="DRAM") as dram_pool,
            tc.tile_pool(name="pr", bufs=2, space="PSUM") as pr,   # 2x [P,1024]
            tc.tile_pool(name="ph", bufs=1, space="PSUM") as ph,   # 4x [P,512]
        ):
            _sm = [0]

            def small_psum(shape, name, dtype=F32, tag=None):
                if tag is None:
                    i = _sm[0] % 4
                    _sm[0] += 1
                    tag = f"bank{i}"
                return ph.tile(shape, dtype, tag=tag, name=name)

            # ---------------- DMAs ----------------
            wq2_s = persist.tile([P, NEC, P], BF16, tag="wq2")
            nc.scalar.dma_start(wq2_s[:], wq2_d[:].rearrange("(c p) d -> p c d", p=P))
            wk2_s = persist.tile([P, NEC, P], BF16, tag="wk2")
            nc.scalar.dma_start(wk2_s[:], wk2_d[:].rearrange("(c p) d -> p c d", p=P))
            wv2_s = persist.tile([P, NEC, P], BF16, tag="wv2")
            nc.scalar.dma_start(wv2_s[:], wv2_d[:].rearrange("(c p) d -> p c d", p=P))
            sc_s = persist.tile([P, 2, NT], BF16, tag="sc")
            nc.scalar.dma_start(sc_s[:], sc_d[:].rearrange("(k p) r -> p k r", p=P))
            ub_s = persist.tile([P, 1], F32, tag="ub")
            nc.scalar.dma_start(ub_s[:], ub2_d[:])
            vb_s = persist.tile([P, 1], F32, tag="vb")
            nc.scalar.dma_start(vb_s[:], vb2_d[:])
            # wkr stacked on partitions: rows 0:64 = head0 d, 64:128 = head1 d
            wkr_s = persist.tile([P, NEC, P], BF16, tag="wkr")
            nc.scalar.dma_start(
                wkr_s[:], wkrT_d[:].rearrange("p (c e) -> p c e", c=NEC)
            )
            wo_s = persist.tile([D, 2, E], BF16, tag="wo")
            nc.scalar.dma_start(wo_s[:], wo2_d[:].rearrange("p (h e) -> p h e", h=2))

            # x^T x-halves + SgA zero/T DMA first (k copies overwrite SgA's
            # k-halves, so those DMAs gate the k projection); history + the
            # rest follow, split across both DMA queues.
            axT = []
            for c in range(NEC):
                axc = persist.tile([P, HpN], BF16, tag=f"axT{c}", name=f"axT{c}")
                q = nc.sync if c < 2 else nc.scalar
                q.dma_start(axc[:, H:], axT_d[c * P : (c + 1) * P, H:])
                axT.append(axc)
            rot_s = persist.tile([P, 4, N], BF16, tag="rot")
            nc.sync.dma_start(rot_s[:, 0, :], rot_d[0:P, :])
            nc.sync.dma_start(rot_s[:, 2, :], rot_d[2 * P : 3 * P, :])
            SgA = []
            HKT = NKT * 4 * P
            for h in range(HEADS_PER_CORE):
                t = persist.tile([P, NKT, 4, P], FP8, tag=f"SgA{h}", name=f"SgA{h}")
                nc.sync.dma_start(
                    t[:, 0 : NKT // 2, :, :],
                    psiA_d[h * HKT : h * HKT + HKT // 2].rearrange(
                        "(t c p) j -> p t c j", p=P, c=4
                    ),
                )
                SgA.append(t)
            for c in range(NEC):
                q = nc.scalar if c < 2 else nc.sync
                q.dma_start(axT[c][:, 0:H], axT_d[c * P : (c + 1) * P, 0:H])
            nc.sync.dma_start(rot_s[:, 1, :], rot_d[P : 2 * P, :])
            nc.sync.dma_start(rot_s[:, 3, :], rot_d[3 * P : 4 * P, :])
            for h in range(HEADS_PER_CORE):
                nc.sync.dma_start(
                    SgA[h][:, NKT // 2 :, :, :],
                    psiA_d[h * HKT + HKT // 2 : (h + 1) * HKT].rearrange(
                        "(t c p) j -> p t c j", p=P, c=4
                    ),
                )
            SgF = persist.tile([P, NKT, 2, P], FP8, tag="SgF")
            nc.scalar.dma_start(
                SgF[:], psiF_d[:].rearrange("(t c p) j -> p t c j", p=P, c=2)
            )

            identb = persist.tile([P, P], BF16, tag="identb")
            make_identity(nc, identb[:])

            # ---------------- persistent compute tiles ----------------
            # M chunks per head: 0=Uhi 1=Whi 2=Ulo 3=Wlo 4=[qhi|chi]
            # 5=[qhi-dup|clo] 6=[qlo|chi-dup] 7=zero-pad
            M = []
            for h in range(HEADS_PER_CORE):
                m = persist.tile([P, 8, NQC, 512], FP8, tag=f"M{h}", name=f"M{h}")
                nc.vector.memset(m[:, 7, :, :], 0.0)
                M.append(m)
            qv_s = persist.tile([P, N], BF16, tag="qv_s")
            vo = []
            for h in range(HEADS_PER_CORE):
                v = persist.tile([P, NKT, 66], BF16, tag=f"vo{h}", name=f"vo{h}")
                nc.gpsimd.memset(v[:, :, 64:66], 0.0)
                nc.gpsimd.memset(v[:, :, 64:65], 1.0)
                vo.append(v)
            numT = []
            numTT = []
            for h in range(HEADS_PER_CORE):
                t = persist.tile([D, N], BF16, tag=f"numT{h}", name=f"numT{h}")
                numT.append(t)
                tt = persist.tile(
                    [P, NS, 65], BF16, tag=f"numTT{h}", name=f"numTT{h}"
                )
                numTT.append(tt)
            out_acc = persist.tile([P, NS, E], BF16, tag="out_acc")
            nbias = persist.tile([P, 1], F32, tag="nbias")
            nc.vector.memset(nbias[:], -SCORE_SHIFT)

            # ---------------- phase A: projections ----------------
            # q projection, both heads packed, emitted chunk-outer so the PE
            # starts as soon as each axT chunk lands
            pqs = [small_psum([P, 512], f"pq{qc}") for qc in range(NQC)]
            for c in range(NEC):
                for qc in range(NQC):
                    nc.tensor.matmul(
                        pqs[qc][:],
                        wq2_s[:, c, :],
                        axT[c][:, H + qc * 512 : H + (qc + 1) * 512],
                        start=(c == 0),
                        stop=(c == NEC - 1),
                    )
            for qc in range(NQC):
                pq = pqs[qc]
                qs = slice(qc * 512, (qc + 1) * 512)
                nc.vector.tensor_scalar_add(qv_s[:, qs], pq[:], vb_s[:])
                for h in range(HEADS_PER_CORE):
                    hp = slice(h * D, (h + 1) * D)
                    nc.vector.tensor_scalar_add(
                        M[h][hp, 4, qc, :], pq[hp, :], ub_s[hp]
                    )
                    nc.vector.scalar_tensor_tensor(
                        M[h][hp, 6, qc, :], pq[hp, :], ub_s[hp],
                        M[h][hp, 4, qc, :], ALU.add, ALU.subtract,
                    )

            def emit_uw_g(h, qc, sfd, ssd):
                hp = slice(h * D, (h + 1) * D)
                qs = slice(qc * 512, (qc + 1) * 512)
                # G: e 0:128 sin-fast + 256:384 cos-fast (sf);
                #    e 128:256 sin-slow + 384:512 cos-slow (ss)
                gf = pr.tile([P, 1024], F32, tag="sp", name="gf")
                nc.tensor.matmul(
                    gf[:, 0:512], wkr_s[hp, 0, :], qv_s[hp, qs],
                    start=True, stop=True,
                )
                nc.tensor.matmul(
                    gf[:, 512:1024], wkr_s[hp, 2, :], qv_s[hp, qs],
                    start=True, stop=True,
                )
                gs = pr.tile([P, 1024], F32, tag="sp", name="gs")
                nc.tensor.matmul(
                    gs[:, 0:512], wkr_s[hp, 1, :], qv_s[hp, qs],
                    start=True, stop=True,
                )
                nc.tensor.matmul(
                    gs[:, 512:1024], wkr_s[hp, 3, :], qv_s[hp, qs],
                    start=True, stop=True,
                )
                nc.scalar.copy(sfd, gf[:])
                nc.scalar.copy(ssd, gs[:])

            def emit_uw_rot(h, qc, sf, ss, usw):
                qs = slice(qc * 512, (qc + 1) * 512)
                # fast half: U = G*cos + Gc*sin ; W = Gc*cos - G*sin
                m1 = mst.tile([P, 512], BF16, tag="m1")
                m2 = mst.tile([P, 512], BF16, tag="m2")
                m3 = mst.tile([P, 512], BF16, tag="m3")
                m4 = mst.tile([P, 512], BF16, tag="m4")
                nc.vector.tensor_mul(m1[:], sf[:, 0:512], rot_s[:, 0, qs])
                nc.vector.tensor_mul(m2[:], sf[:, 512:1024], rot_s[:, 2, qs])
                nc.vector.tensor_mul(m3[:], sf[:, 512:1024], rot_s[:, 0, qs])
                nc.vector.tensor_mul(m4[:], sf[:, 0:512], rot_s[:, 2, qs])
                ubf = mst.tile([P, 512], BF16, tag="ubf")
                wbf = mst.tile([P, 512], BF16, tag="wbf")
                nc.gpsimd.tensor_add(ubf[:], m1[:], m2[:])
                nc.gpsimd.tensor_sub(wbf[:], m3[:], m4[:])
                nc.vector.tensor_copy(M[h][:, 0, qc, :], ubf[:])
                nc.gpsimd.tensor_copy(M[h][:, 1, qc, :], wbf[:])
                nc.gpsimd.tensor_sub(M[h][:, 2, qc, :], ubf[:], M[h][:, 0, qc, :])
                nc.vector.tensor_sub(M[h][:, 3, qc, :], wbf[:], M[h][:, 1, qc, :])
                # slow half: rotate; compression happens in emit_uw_cheb
                m5 = mst.tile([P, 512], BF16, tag="m1", name="m5")
                m6 = mst.tile([P, 512], BF16, tag="m2", name="m6")
                m7 = mst.tile([P, 512], BF16, tag="m3", name="m7")
                m8 = mst.tile([P, 512], BF16, tag="m4", name="m8")
                nc.vector.tensor_mul(m5[:], ss[:, 0:512], rot_s[:, 1, qs])
                nc.vector.tensor_mul(m6[:], ss[:, 512:1024], rot_s[:, 3, qs])
                nc.vector.tensor_mul(m7[:], ss[:, 512:1024], rot_s[:, 1, qs])
                nc.vector.tensor_mul(m8[:], ss[:, 0:512], rot_s[:, 3, qs])
                nc.gpsimd.tensor_add(usw[:, 0, :], m5[:], m6[:])
                nc.gpsimd.tensor_sub(usw[:, 1, :], m7[:], m8[:])

            def emit_uw_cheb(h, qc, usw):
                # cheb coefs land on the head's opposite partition half
                po = (1 - h) * D
                cs = slice(po, po + NT)
                pc = small_psum([P, 512], "pc")
                for k in range(2):
                    nc.tensor.matmul(
                        pc[cs, :], sc_s[:, k, :], usw[:, k, :],
                        start=(k == 0), stop=(k == 1),
                    )
                nc.scalar.copy(M[h][cs, 4, qc, :], pc[cs, :])
                nc.vector.tensor_sub(
                    M[h][cs, 5, qc, :], pc[cs, :], M[h][cs, 4, qc, :]
                )

            def emit_k(kc):
                pk = small_psum([P, 512], "pk")
                for c in range(NEC):
                    nc.tensor.matmul(
                        pk[:],
                        wk2_s[:, c, :],
                        axT[c][:, kc * 512 : (kc + 1) * 512],
                        start=(c == 0),
                        stop=(c == NEC - 1),
                    )
                ks = slice(4 * kc, 4 * kc + 4)
                for h in range(HEADS_PER_CORE):
                    hp = slice(h * D, (h + 1) * D)
                    pkv = pk[hp, :].rearrange("p (t j) -> p t j", j=P)
                    nc.scalar.copy(SgA[h][hp, ks, 0, :], pkv)
                    nc.vector.tensor_sub(
                        SgA[h][hp, ks, 1, :], pkv, SgA[h][hp, ks, 0, :]
                    )

            def emit_v(h, g, tag=None):
                hs = slice(h * D, (h + 1) * D)
                pv = small_psum([P, 512], "pv", tag=tag)
                for k8 in range(8):
                    kt = g * 8 + k8
                    for c in range(NEC):
                        nc.tensor.matmul(
                            pv[:, k8 * D : (k8 + 1) * D],
                            axT[c][:, kt * P : (kt + 1) * P],
                            wv2_s[:, c, hs],
                            start=(c == 0),
                            stop=(c == NEC - 1),
                        )
                nc.vector.tensor_copy(
                    vo[h][:, g * 8 : (g + 1) * 8, 0:D],
                    pv[:].rearrange("p (t d) -> p t d", d=D),
                )

            # h0 UW fully in phase A (streaming); h1's G matmuls too, but the
            # copies land in persistent tiles recycled from axT tags so h1's
            # rotation (engine-only) can run during h0's attention.
            h1buf = []
            for qc in range(NQC):
                t = persist.tile(
                    [P, 6, 512], BF16, tag=f"axT{qc}", name=f"h1buf{qc}"
                )
                h1buf.append(t)

            for u in range(NQC):
                sf = gst.tile([P, 1024], BF16, tag="sf")
                ss = gst.tile([P, 1024], BF16, tag="ss")
                usw = mst.tile([P, 2, 512], BF16, tag="usw")
                emit_uw_g(0, u, sf[:], ss[:])
                emit_uw_rot(0, u, sf, ss, usw)
                emit_uw_cheb(0, u, usw)
                emit_k(2 * u)
            for u in range(NQC):
                emit_uw_g(
                    1, u,
                    h1buf[u][:, 0:2, :].rearrange("p a b -> p (a b)"),
                    h1buf[u][:, 2:4, :].rearrange("p a b -> p (a b)"),
                )
                emit_k(2 * u + 1)
                emit_v(0, u)
                emit_v(1, u)

            # dups via DMA (off-engine): M chunk 5 q-half <- chunk 4 q-half;
            # M chunk 6 cheb-half <- chunk 4 cheb-half (h0 now, h1 after its
            # cheb block); SgA chunk 2 <- chunk 0
            for h in range(HEADS_PER_CORE):
                hp = slice(h * D, (h + 1) * D)
                nc.sync.dma_start(M[h][hp, 5, :, :], M[h][hp, 4, :, :])
                nc.sync.dma_start(SgA[h][hp, :, 2, :], SgA[h][hp, :, 0, :])
            cs0 = slice(D, D + NT)
            nc.sync.dma_start(M[0][cs0, 6, :, :], M[0][cs0, 4, :, :])

            # ---------------- phase B: attention ----------------
            _expctr = [0]

            def emit_pair(h, pi, avv, pend):
                kt0 = 2 * pi
                for qc in range(NQC):
                    ps = pr.tile([P, 1024], F32, tag="sp", name="ps")
                    for half in range(2):
                        kt = kt0 + half
                        os = slice(half * 512, (half + 1) * 512)
                        nc.tensor.matmul(
                            ps[:, os], SgF[:, kt, :, :], M[h][:, 0:2, qc, :],
                            start=True, stop=False, perf_mode=DR,
                        )
                        nc.tensor.matmul(
                            ps[:, os], SgF[:, kt, :, :], M[h][:, 2:4, qc, :],
                            start=False, stop=False, perf_mode=DR,
                        )
                        nc.tensor.matmul(
                            ps[:, os], SgA[h][:, kt, 0:2, :], M[h][:, 4:6, qc, :],
                            start=False, stop=False, perf_mode=DR,
                        )
                        nc.tensor.matmul(
                            ps[:, os], SgA[h][:, kt, 2:4, :], M[h][:, 6:8, qc, :],
                            start=False, stop=True, perf_mode=DR,
                        )
                    if qc in pend:
                        ppi, pE = pend.pop(qc)
                        for j in range(2):
                            for qt in range(4):
                                qg = qc * 4 + qt
                                bk, sl = divmod(qg, 6)
                                first = ppi == 0 and j == 0 and sl == 0
                                last = (
                                    ppi == NPAIR - 1 and j == 1
                                    and (qg in (5, 11, 15))
                                )
                                nc.tensor.matmul(
                                    avv[bk][:, sl, :],
                                    pE[:, j, qt * P : (qt + 1) * P],
                                    vo[h][:, 2 * ppi + j, 0:65],
                                    start=first, stop=last,
                                    skip_group_check=True,
                                )
                    et = est.tile([P, 2, 512], BF16, tag="E")
                    if (_expctr[0] % EXP_MOD) < EXP_DVE:
                        # Schraudolph: int16 bits = 128*(log2e*(s/8 - c) + 127)
                        nc.vector.tensor_scalar(
                            et[:].bitcast(I16), ps[:],
                            0.125 * P * LOG2E,
                            P * 127.0 - SCORE_SHIFT * P * LOG2E - 8.5,
                            ALU.mult, ALU.add,
                        )
                    else:
                        nc.scalar.activation(
                            et[:], ps[:], AF.Exp, scale=0.125, bias=nbias[:]
                        )
                    _expctr[0] += 1
                    pend[qc] = (pi, et)

            def emit_av_flush(h, avv, pend):
                for qc, (ppi, pE) in sorted(pend.items()):
                    for j in range(2):
                        for qt in range(4):
                            qg = qc * 4 + qt
                            bk, sl = divmod(qg, 6)
                            nc.tensor.matmul(
                                avv[bk][:, sl, :],
                                pE[:, j, qt * P : (qt + 1) * P],
                                vo[h][:, 2 * ppi + j, 0:65],
                                start=False,
                                stop=(
                                    ppi == NPAIR - 1 and j == 1
                                    and (qg in (5, 11, 15))
                                ),
                                skip_group_check=True,
                            )
                pend.clear()

            def emit_z(h, avv):
                # av is query-major with the ones-column z in slot 64; copy to
                # sbuf, reciprocal per-partition, transpose values to d-major
                ntt = numTT[h]
                nc.vector.tensor_copy(ntt[:, 0:6, :], avv[0][:])
                nc.vector.tensor_copy(ntt[:, 6:12, :], avv[1][:])
                nc.vector.tensor_copy(ntt[:, 12:16, :], avv[2][:])
                zrec = persist.tile([P, NS], F32, tag=f"zrec{h}", name=f"zrec{h}")
                nc.vector.reciprocal(zrec[:], ntt[:, :, 64])
                for s in range(NS):
                    pz = small_psum([D, P], "pz", BF16)
                    nc.tensor.transpose(pz[:], ntt[:, s, 0:D], identb[:])
                    nc.scalar.copy(numT[h][:, s * P : (s + 1) * P], pz[:])
                return zrec

            def emit_out_pair(h, sp, zrec):
                pp = pr.tile([P, 1024], F32, tag="sp", name="pp")
                for j in range(2):
                    s = sp + j
                    nc.tensor.matmul(
                        pp[:, j * 512 : (j + 1) * 512],
                        numT[h][0:D, s * P : (s + 1) * P], wo_s[:, h, :],
                        start=True, stop=True,
                    )
                for j in range(2):
                    s = sp + j
                    pj = pp[:, j * 512 : (j + 1) * 512]
                    nc.scalar.activation(
                        out_acc[:, s, :], pj, AF.Copy, scale=zrec[:, s : s + 1]
                    )

            # h0 attention with h1's rotation (engine-only) interleaved
            # av[j]: value accumulators for qtiles 8j..8j+7; avz: denominators
            av0 = [
                ph.tile([P, 6 if j < 2 else 4, 65], F32, tag=f"bank{j}",
                        name=f"av0{j}")
                for j in range(3)
            ]
            pend0 = {}
            for pi in range(NPAIR):
                emit_pair(0, pi, av0, pend0)
                if pi in (1, 3, 5, 7):
                    u = (pi - 1) // 2
                    emit_uw_rot(
                        1, u, h1buf[u][:, 0:2, :].rearrange("p a b -> p (a b)"),
                        h1buf[u][:, 2:4, :].rearrange("p a b -> p (a b)"),
                        h1buf[u][:, 4:6, :],
                    )

            emit_av_flush(0, av0, pend0)
            zrec0 = emit_z(0, av0)
            # transition: h1 cheb compression + its dup DMA
            for u in range(NQC):
                emit_uw_cheb(1, u, h1buf[u][:, 4:6, :])
            cs1 = slice(0, NT)
            nc.sync.dma_start(M[1][cs1, 6, :, :], M[1][cs1, 4, :, :])

            # h1 attention with h0's output projection interleaved
            av1 = [
                ph.tile([P, 6 if j < 2 else 4, 65], F32, tag=f"bank{j}",
                        name=f"av1{j}")
                for j in range(3)
            ]
            pend1 = {}
            for pi in range(NPAIR):
                emit_pair(1, pi, av1, pend1)
                if pi >= 1 and pi % 2 == 1:
                    emit_out_pair(0, (pi // 2) * 2, zrec0)
            emit_av_flush(1, av1, pend1)
            zrec1 = emit_z(1, av1)
            # h1 out: all matmuls first (buffered over pr + ph banks), then
            # the zrec-gated stores drain as buffers free
            h1bufs = []
            for i, sp in enumerate(range(0, NS, 2)):
                if i % 2 == 0:
                    pp = pr.tile([P, 1024], F32, tag="sp", name="pp")
                    slots = (pp[:, 0:512], pp[:, 512:1024])
                else:
                    sa = small_psum([P, 512], "poa")
                    sb = small_psum([P, 512], "pob")
                    slots = (sa[:], sb[:])
                for j in range(2):
                    s = sp + j
                    nc.tensor.matmul(
                        slots[j], numT[1][0:D, s * P : (s + 1) * P], wo_s[:, 1, :],
                        start=True, stop=True,
                    )
                h1bufs.append((sp, slots))
            for sp, slots in h1bufs:
                for j in range(2):
                    s = sp + j
                    nc.vector.scalar_tensor_tensor(
                        out_acc[:, s, :], slots[j], zrec1[:, s : s + 1],
                        out_acc[:, s, :], ALU.mult, ALU.add,
                    )
                    nc.sync.dma_start(
                        out_d[:].rearrange("(s p) e -> p s e", p=P)[:, s, :],
                        out_acc[:, s, :],
                    )

    nc.compile()
    return nc


_NC_CACHE = None


def _get_program():
    global _NC_CACHE
    if _NC_CACHE is None:
        _NC_CACHE = build_program()
    return _NC_CACHE


def _fp8_hl(x):
    hi = np.clip(np.asarray(x, np.float32), -240, 240).astype(ml_dtypes.float8_e4m3)
    lo = np.clip(
        np.asarray(x, np.float32) - hi.astype(np.float32), -240, 240
    ).astype(ml_dtypes.float8_e4m3)
    return hi, lo


def make_in_maps(x, history, w_q, w_k, w_v, w_kr, w_o, u_bias, v_bias):
    bf = ml_dtypes.bfloat16
    all_x = np.concatenate([history, x], axis=1)  # [B, HpN, E]

    inv_freq = 1.0 / (10000.0 ** (np.arange(0, E, 2, dtype=np.float64) / E))  # [256]
    ang_f = np.outer(inv_freq[:128], np.arange(HpN, dtype=np.float64) - H)
    xn = (np.arange(HpN, dtype=np.float64) - H) / 2048.0
    T = np.polynomial.chebyshev.chebvander(xn, NT - 1)  # [HpN, NT]
    ang_s = np.outer(xn * 2048.0, inv_freq[128:256])  # [HpN, 128]
    tgt = np.concatenate([np.sin(ang_s), np.cos(ang_s)], axis=1)  # [HpN, 256]
    coef, *_ = np.linalg.lstsq(T, tgt, rcond=None)  # [NT, 256]
    sc = np.ascontiguousarray(coef.T)  # [256, NT]: rows 0-127 sin, 128-255 cos

    sin_hi, _ = _fp8_hl(np.sin(ang_f))
    cos_hi, _ = _fp8_hl(np.cos(ang_f))
    T_hi, T_lo = _fp8_hl(T.T)  # [NT, HpN]
    # SgF: [kt, 2, 128, 128] chunks [sin_hi, cos_hi]
    psiF = np.stack(
        [sin_hi.astype(np.float32), cos_hi.astype(np.float32)], axis=0
    )  # [2, 128, HpN]
    psiF = np.ascontiguousarray(
        psiF.reshape(2, P, NKT, P).transpose(2, 0, 1, 3)
    ).reshape(NKT * 2 * P, P)
    # SgA per head: chunks [ [k|Thi], [k|Thi], [k|Tlo], 0 ]; k-half zeros here.
    # k at the head's partition half (h0 -> 0:64), T at the other half
    psiA = np.zeros((HEADS_PER_CORE, 4, P, HpN), np.float32)
    Thif = T_hi.astype(np.float32)
    Tlof = T_lo.astype(np.float32)
    for h in range(HEADS_PER_CORE):
        tp = (1 - h) * D
        psiA[h, 0, tp : tp + NT] = Thif
        psiA[h, 1, tp : tp + NT] = Thif
        psiA[h, 2, tp : tp + NT] = Tlof
    psiA = np.ascontiguousarray(
        psiA.reshape(HEADS_PER_CORE, 4, P, NKT, P).transpose(0, 3, 1, 2, 4)
    ).reshape(HEADS_PER_CORE * NKT * 4 * P, P)

    ang_b = np.outer(inv_freq, np.arange(N, dtype=np.float64))  # [256, N]
    rot = np.ascontiguousarray(
        np.concatenate([np.cos(ang_b), np.sin(ang_b)]).astype(bf)
    )  # [512, N]: rows 0:128 cos-fast, 128:256 cos-slow, 256:384 sin-fast, ...

    clip8 = lambda a: np.clip(a, -240, 240).astype(ml_dtypes.float8_e4m3)

    in_maps = []
    for c in range(N_CORES):
        b = c // 4
        h0 = HEADS_PER_CORE * (c % 4)
        axT = np.ascontiguousarray(all_x[b].T).astype(bf)
        wq2 = np.concatenate([w_q[h0], w_q[h0 + 1]], axis=1).astype(bf)  # [E, 128]
        wk2 = np.concatenate([w_k[h0], w_k[h0 + 1]], axis=1).astype(bf)
        wv2 = np.concatenate([w_v[h0], w_v[h0 + 1]], axis=1).astype(bf)
        wkrT = np.concatenate(
            [w_kr[h0].T, w_kr[h0 + 1].T], axis=0
        ).astype(bf)  # [128, E]: rows 0:64 = head0 (d), 64:128 = head1
        wo2 = np.stack([w_o[h0], w_o[h0 + 1]], axis=1).reshape(D, 2 * E).astype(bf)
        in_maps.append(
            {
                "axT": axT,
                "rot": rot,
                "psiF": clip8(psiF),
                "psiA": clip8(psiA),
                "sc": np.ascontiguousarray(sc).astype(bf),
                "wq2": np.ascontiguousarray(wq2),
                "wk2": np.ascontiguousarray(wk2),
                "wv2": np.ascontiguousarray(wv2),
                "wkrT": np.ascontiguousarray(wkrT),
                "wo2": np.ascontiguousarray(wo2),
                "ub2": np.ascontiguousarray(
                    np.concatenate([u_bias[h0], u_bias[h0 + 1]]).reshape(P, 1)
                ).astype(np.float32),
                "vb2": np.ascontiguousarray(
                    np.concatenate([v_bias[h0], v_bias[h0 + 1]]).reshape(P, 1)
                ).astype(np.float32),
            }
        )
    return in_maps


def run(inputs, trace=False, **kw):
    from concourse.bass_utils import run_bass_kernel_spmd

    nc = _get_program()
    in_maps = make_in_maps(
        np.asarray(inputs["x"], np.float32),
        np.asarray(inputs["history"], np.float32),
        np.asarray(inputs["w_q"], np.float32),
        np.asarray(inputs["w_k"], np.float32),
        np.asarray(inputs["w_v"], np.float32),
        np.asarray(inputs["w_kr"], np.float32),
        np.asarray(inputs["w_o"], np.float32),
        np.asarray(inputs["u_bias"], np.float32),
        np.asarray(inputs["v_bias"], np.float32),
    )
    res = run_bass_kernel_spmd(nc, in_maps, list(range(N_CORES)), trace=trace, **kw)
    out = np.zeros((B, N, E), np.float32)
    for c in range(N_CORES):
        out[c // 4] += res.results[c]["out"].astype(np.float32).reshape(N, E)
    return out, res


def kernel(**inputs):
    # mask is all ones (per the problem spec), so score masking is a no-op
    # and the tensor is ignored.
    out, _ = run(inputs, trace=False)
    return out
